# revision 8
# baseline (speedup 1.0000x reference)
"""GNN message-passing kernel for trn2 v2.

h = relu(BN(s1 @ W_pre)); agg = segment_sum(h[src], dst);
out = relu((1-b)*support + b*support@W_op), support = 0.9*(h+agg) + 0.1*x_0.

v2 design:
- Gram (BN stats) sharded 8-way + AllReduce of the 257-col Gram matrix.
- Phase B (h table) replicated per core; table is node-linear: node v at
  (partition v//392, col v%392), flat 512B row index = (v//392)*393 + v%392.
  Col 392 of every partition is a zero row (gather padding target).
- Phase C: slot-identity aggregation. Dst nodes globally sorted by
  (lo-degree, hi-degree), dealt round-robin to cores in tiles of 128.
  Each tile gathers K_lo + K_hi slot-chunks (padded with zero rows); the
  segment sum is plain PSUM accumulation with a constant identity lhsT —
  no per-chunk selection matrices, no per-chunk weight loads.
"""
import math
import numpy as np
import ml_dtypes

import concourse.bass as bass
import concourse.bacc as bacc
import concourse.mybir as mybir
from concourse.tile import TileContext

BF16 = mybir.dt.bfloat16
F32 = mybir.dt.float32
I16 = mybir.dt.int16

ALPHA = 0.1
LAMBDA = 0.5
BN_EPS = 1e-5
BETA_C = float(np.log(LAMBDA / 1.0 + 1.0))   # 0.405465
W_OP_SCALE = BETA_C / (1.0 - BETA_C)
OUT_SCALE = 1.0 - BETA_C

PCOLS = 392          # data cols per partition (node v -> (v//392, v%392))
TCOLS = PCOLS + 1    # + zero col
NSLOT = 128 * PCOLS  # 50176 node slots
SG = 1               # tiles per gather supercall


class Prob:
    def __init__(self, N, E, C, HID, n_cores):
        self.N, self.E, self.C, self.HID, self.n_cores = N, E, C, HID, n_cores
        assert C == 256 and HID == 256 and N == 50000
        self.gtiles = NSLOT // 128                  # 392 global dst tiles
        self.tiles = self.gtiles // n_cores         # 49 local tiles
        self.gchunks = self.gtiles                  # gram chunks (node-major)
        self.gsh = self.gchunks // n_cores          # 49 gram chunks per core


def host_prep(prob, s1, x_0, edge_index):
    p = prob
    N, M = p.N, p.n_cores
    s1 = np.asarray(s1, dtype=np.float32)
    x_0 = np.asarray(x_0, dtype=np.float32)
    src = np.asarray(edge_index[0], dtype=np.int64)
    dst = np.asarray(edge_index[1], dtype=np.int64)
    # self edges (GIN +h term)
    src = np.concatenate([src, np.arange(N, dtype=np.int64)])
    dst = np.concatenate([dst, np.arange(N, dtype=np.int64)])

    half = (src >= 25088).astype(np.int64)          # table partition >= 64
    # per-dst lo/hi degrees
    L = np.bincount(dst[half == 0], minlength=N)
    H = np.bincount(dst[half == 1], minlength=N)
    # dst slot assignment: pads first, then sorted by (L, H)
    snakeH = np.where(L % 2 == 0, H, -H)            # snake order: H continuous
    order = np.lexsort((snakeH, L))                 # N real dsts sorted
    dst_slot = np.full(N, -1, np.int64)
    dst_slot[order] = 176 + np.arange(N)            # slots 0..175 are pads
    # slot -> (global tile, lane); global tile g -> core g%M, local tile g//M
    # edge placement: slot idx within (dst, half)
    eorder = np.lexsort((src, half, dst))
    ds, hs, ss = dst[eorder], half[eorder], src[eorder]
    # cumcount within each (dst, half) run
    grp = ds * 2 + hs
    changes = np.empty(len(grp), bool)
    changes[0] = True
    np.not_equal(grp[1:], grp[:-1], out=changes[1:])
    run_starts = np.flatnonzero(changes)
    slot_in_grp = np.arange(len(grp)) - np.repeat(run_starts, np.diff(
        np.append(run_starts, len(grp))))
    gslot = dst_slot[ds]
    gtile = gslot >> 7
    lane = gslot & 127
    core = gtile % M
    ltile = gtile // M

    # K per local tile (shared across cores)
    maxL = np.zeros((M, p.gtiles // M), np.int64)
    maxH = np.zeros((M, p.gtiles // M), np.int64)
    lmax = np.bincount(dst_slot, weights=L.astype(np.float64),
                       minlength=NSLOT)  # per slot degree (scatter)
    lmax_s = np.zeros(NSLOT, np.int64)
    lmax_s[dst_slot] = L
    hmax_s = np.zeros(NSLOT, np.int64)
    hmax_s[dst_slot] = H
    tl = lmax_s.reshape(p.gtiles, 128).max(axis=1)  # per global tile max L
    th = hmax_s.reshape(p.gtiles, 128).max(axis=1)
    K_lo = tl.reshape(p.gtiles // M, M).max(axis=1)  # per local tile (max cores)
    K_hi = th.reshape(p.gtiles // M, M).max(axis=1)
    tiles = p.tiles

    # flat row index in table (node-linear)
    flat = (ss // PCOLS) * TCOLS + (ss % PCOLS)
    is_lo = hs == 0
    # within-call positions
    # per-core idx arrays: for each local tile, lo block K_lo[t]*128 then built
    # at supercall granularity: supercall k covers tiles [k*SG,(k+1)*SG)
    nsc = (tiles + SG - 1) // SG
    # column offsets
    lo_off = np.zeros(tiles + 1, np.int64)
    hi_off = np.zeros(tiles + 1, np.int64)
    np.cumsum(K_lo, out=lo_off[1:])
    np.cumsum(K_hi, out=hi_off[1:])
    tot_lo = int(lo_off[-1])
    tot_hi = int(hi_off[-1])
    # prefill with zero-row padding
    lane_grid = np.tile(np.arange(128), tot_lo)     # not lane-aligned; fix below
    idx_lo = np.empty((M, tot_lo * 128), np.int16)
    idx_hi = np.empty((M, tot_hi * 128), np.int16)
    pad_pat = ((np.arange(128) % 64) * TCOLS + PCOLS).astype(np.int16)
    idx_lo[:] = np.tile(pad_pat, tot_lo)[None, :]
    idx_hi[:] = np.tile(pad_pat, tot_hi)[None, :]
    # scatter edges
    pos_lo = (lo_off[ltile] + slot_in_grp) * 128 + lane
    pos_hi = (hi_off[ltile] + slot_in_grp) * 128 + lane
    lo_m = is_lo
    hi_m = ~is_lo
    # (vectorized per core)
    for m in range(M):
        cm = core == m
        sel = cm & lo_m
        idx_lo[m, pos_lo[sel]] = flat[sel].astype(np.int16)
        sel = cm & hi_m
        idx_hi[m, pos_hi[sel]] = (flat[sel] - 64 * TCOLS).astype(np.int16)
    # supercall idx layout: per supercall, lo idx then separately hi list;
    # calls interleaved lo/hi. Build the wrapped [128, cols] layout.
    def wrap(a):
        # [n] -> [16, n/16] -> tile to 128 partitions
        n = len(a)
        lay = a.reshape(n // 16, 16).T
        return np.tile(lay, (8, 1))
    cols_lo = []
    cols_hi = []
    sc_lo = []  # (n_chunks per supercall)
    sc_hi = []
    for k in range(nsc):
        t0, t1 = k * SG, min(tiles, (k + 1) * SG)
        a0, a1 = int(lo_off[t0]) * 128, int(lo_off[t1]) * 128
        b0, b1 = int(hi_off[t0]) * 128, int(hi_off[t1]) * 128
        sc_lo.append((a1 - a0) // 128)
        sc_hi.append((b1 - b0) // 128)
        cols_lo.append((a0, a1))
        cols_hi.append((b0, b1))
    idx_cat = []
    for m in range(M):
        parts = []
        for k in range(nsc):
            a0, a1 = cols_lo[k]
            b0, b1 = cols_hi[k]
            if a1 > a0:
                parts.append(wrap(idx_lo[m, a0:a1]))
            if b1 > b0:
                parts.append(wrap(idx_hi[m, b0:b1]))
        idx_cat.append(np.concatenate(parts, axis=1).astype(np.int16))
    idx_all = np.stack(idx_cat)                     # [M, 128, totcols]

    # ---- phase A inputs: node-major chunked s1 extended (257 cols) ----
    npad = p.gchunks * 128                          # 50176
    s1e = np.zeros((npad, 257), np.float32)
    s1e[:N, :256] = s1
    s1e[:N, 256] = 1.0
    s1e = s1e.reshape(p.gchunks, 128, 257).transpose(1, 0, 2)  # [128, ch, 257]
    s1e = np.ascontiguousarray(s1e).astype(ml_dtypes.bfloat16)
    # per-core slice of 49 chunks
    s1e_sh = [np.ascontiguousarray(
        s1e[:, m * p.gsh:(m + 1) * p.gsh, :]).reshape(128, -1)
        for m in range(M)]

    # ---- phase B input: s1T with cols reordered to table order ----
    # col (c*128 + q) = s1[node q*392 + c]  (0 for pads)
    nodes = (np.arange(NSLOT).reshape(PCOLS, 128, order='F'))  # [c, q]? build explicitly
    cgrid, qgrid = np.meshgrid(np.arange(PCOLS), np.arange(128), indexing='ij')
    node_of = qgrid * PCOLS + cgrid                  # [c, q]
    s1T = np.zeros((256, NSLOT), np.float32)
    valid = node_of < N
    s1T[:, (cgrid * 128 + qgrid)[valid]] = s1[node_of[valid]].T
    s1T = np.ascontiguousarray(s1T).astype(ml_dtypes.bfloat16)

    # ---- x0 per core (bf16) ----
    slot_node = np.full(NSLOT, -1, np.int64)
    slot_node[dst_slot] = np.arange(N)
    x0s = np.zeros((M, 128, p.tiles * 256), np.float32)
    out_map = np.full((M, p.tiles, 128), -1, np.int64)
    for m in range(M):
        gt = np.arange(p.tiles) * M + m
        sl = (gt[:, None] * 128 + np.arange(128)[None, :])      # [tiles, 128]
        nd = slot_node[sl]                                       # [tiles, 128]
        out_map[m] = nd
        ok = nd >= 0
        # x0s[m, lane, t*256: (t+1)*256] = x_0[node]
        ti, la = np.nonzero(ok)
        vals = x_0[nd[ok]]                                       # [k, 256]
        x0s[m, la[:, None], (ti[:, None] * 256 + np.arange(256)[None, :])] = vals
    x0s = x0s.astype(ml_dtypes.bfloat16)

    identB = np.eye(128, dtype=np.float32).astype(ml_dtypes.bfloat16)
    ident = np.eye(128, dtype=np.float32)
    ones1 = np.ones((1, 128), np.float32)
    onesc = np.ones((128, 1), np.float32)

    meta = dict(K_lo=K_lo, K_hi=K_hi, sc_lo=sc_lo, sc_hi=sc_hi,
                lo_off=lo_off, hi_off=hi_off,
                totcols=idx_all.shape[2], out_map=out_map)
    shared = dict(s1T=s1T, identB=identB, ident=ident, ones1=ones1,
                  onesc=onesc)
    in_maps = []
    for m in range(M):
        d = dict(shared)
        d["s1e"] = s1e_sh[m]
        d["idxall"] = idx_all[m]
        d["x0s"] = np.ascontiguousarray(x0s[m])
        in_maps.append(d)
    return in_maps, meta


def build_kernel(prob, meta, W_pre, gamma, beta_bn, W_op, nloop=1):
    p = prob
    C, HID = p.C, p.HID
    K_lo, K_hi = meta["K_lo"], meta["K_hi"]
    lo_off, hi_off = meta["lo_off"], meta["hi_off"]
    sc_lo, sc_hi = meta["sc_lo"], meta["sc_hi"]
    totcols = meta["totcols"]
    tiles = p.tiles
    nsc = (tiles + SG - 1) // SG

    nc = bacc.Bacc("TRN2", target_bir_lowering=False, debug=False,
                   num_devices=p.n_cores, num_swdge_queues=4)
    t_s1e = nc.dram_tensor("s1e", [128, p.gsh * (C + 1)], BF16, kind="ExternalInput")
    t_s1T = nc.dram_tensor("s1T", [C, NSLOT], BF16, kind="ExternalInput")
    t_wpre = nc.dram_tensor("wpre", [C, HID], F32, kind="ExternalInput")
    t_gamma = nc.dram_tensor("gamma", [1, HID], F32, kind="ExternalInput")
    t_beta = nc.dram_tensor("beta", [1, HID], F32, kind="ExternalInput")
    t_wop = nc.dram_tensor("wop", [HID, HID], F32, kind="ExternalInput")
    t_x0 = nc.dram_tensor("x0s", [128, tiles * HID], BF16, kind="ExternalInput")
    t_idx = nc.dram_tensor("idxall", [128, totcols], I16, kind="ExternalInput")
    t_idB = nc.dram_tensor("identB", [128, 128], BF16, kind="ExternalInput")
    t_ident = nc.dram_tensor("ident", [128, 128], F32, kind="ExternalInput")
    t_ones1 = nc.dram_tensor("ones1", [1, 128], F32, kind="ExternalInput")
    t_onesc = nc.dram_tensor("onesc", [128, 1], F32, kind="ExternalInput")
    t_out = nc.dram_tensor("out", [128, tiles * HID], F32, kind="ExternalOutput")
    t_h = nc.dram_tensor("h_tab", [128, TCOLS * HID], BF16)
    cc_in = nc.dram_tensor("cc_in", [256, C + 1], F32)
    cc_out = nc.dram_tensor("cc_out", [256, C + 1], F32, addr_space="Shared")

    tc1 = TileContext(nc)
    with tc1 as tc:
        with (tc.tile_pool(name="const", bufs=1) as cpool,
              tc.tile_pool(name="s1in", bufs=2) as apool,
              tc.tile_pool(name="span", bufs=2) as spool,
              tc.tile_pool(name="hout", bufs=3) as hpool,
              tc.tile_pool(name="psA", bufs=1, space="PSUM") as psA,
              tc.tile_pool(name="psZ", bufs=3, space="PSUM") as psZ,
              tc.tile_pool(name="small", bufs=1) as smpool):
            w_f32 = []
            for r in range(2):
                w = cpool.tile([128, HID], F32, tag=f"wf{r}")
                nc.sync.dma_start(out=w[:], in_=t_wpre[r * 128:(r + 1) * 128, :])
                w_f32.append(w)
            gamma_sb = cpool.tile([1, HID], F32, tag="gm")
            nc.sync.dma_start(out=gamma_sb[:], in_=t_gamma[:])
            beta_sb = cpool.tile([1, HID], F32, tag="bt")
            nc.sync.dma_start(out=beta_sb[:], in_=t_beta[:])
            ones1_sb = cpool.tile([1, 128], F32, tag="on")
            nc.sync.dma_start(out=ones1_sb[:], in_=t_ones1[:])
            onesc_sb = cpool.tile([128, 1], F32, tag="onc")
            nc.sync.dma_start(out=onesc_sb[:], in_=t_onesc[:])
            wop_f32 = []
            for r in range(2):
                w = cpool.tile([128, HID], F32, tag=f"wo{r}")
                nc.sync.dma_start(out=w[:], in_=t_wop[r * 128:(r + 1) * 128, :])
                wop_f32.append(w)

            def phaseAB():
                # ---- phase A: sharded Gram ----
                gps = [psA.tile([128, C + 1], F32, tag=f"g{r}", name=f"gps{r}")
                       for r in range(2)]
                ASPAN = 25
                CW = C + 1
                nasp = math.ceil(p.gsh / ASPAN)
                for s in range(nasp):
                    j0 = s * ASPAN
                    j1 = min(p.gsh, j0 + ASPAN)
                    s1t = apool.tile([128, ASPAN * CW], BF16, name="s1span")
                    nc.sync.dma_start(out=s1t[:, :(j1 - j0) * CW],
                                      in_=t_s1e[:, j0 * CW:j1 * CW])
                    for j in range(j0, j1):
                        co = (j - j0) * CW
                        for r in range(2):
                            nc.tensor.matmul(gps[r][:],
                                             lhsT=s1t[:, co + r * 128:co + r * 128 + 128],
                                             rhs=s1t[:, co:co + CW],
                                             start=(j == 0), stop=(j == p.gsh - 1))
                # partial gram -> DRAM -> AllReduce -> SBUF
                gpart = []
                for r in range(2):
                    g = smpool.tile([128, C + 1], F32, tag=f"gp{r}")
                    nc.vector.tensor_copy(out=g[:], in_=gps[r][:])
                    gpart.append(g)
                    nc.sync.dma_start(out=cc_in[r * 128:(r + 1) * 128, :], in_=g[:])
                nc.gpsimd.collective_compute(
                    "AllReduce", mybir.AluOpType.add,
                    replica_groups=[list(range(p.n_cores))],
                    ins=[cc_in[:]], outs=[cc_out[:]])
                g_sb = []
                for r in range(2):
                    g = smpool.tile([128, C + 1], F32, tag=f"gsb{r}")
                    nc.sync.dma_start(out=g[:], in_=cc_out[r * 128:(r + 1) * 128, :])
                    g_sb.append(g)

                # ---- stats finalize (baseline math on global gram) ----
                psB = psZ.tile([128, HID], F32, tag="zb")
                b_sb = []
                for r in range(2):
                    for k in range(2):
                        nc.tensor.matmul(psB[:], lhsT=g_sb[k][:, r * 128:(r + 1) * 128],
                                         rhs=w_f32[k][:], start=(k == 0), stop=(k == 1))
                    b = smpool.tile([128, HID], F32, tag=f"bsb{r}")
                    nc.vector.tensor_copy(out=b[:], in_=psB[:])
                    b_sb.append(b)

                def psum_colsum(tiles_in, tag):
                    acc = smpool.tile([128, HID], F32, tag=tag)
                    nc.vector.tensor_tensor(out=acc[:], in0=tiles_in[0][:],
                                            in1=tiles_in[1][:], op=mybir.AluOpType.add)
                    ps = psA.tile([1, HID], F32, tag="zred", name=f"ps_{tag}")
                    nc.tensor.matmul(ps[:], lhsT=onesc_sb[:], rhs=acc[:],
                                     start=True, stop=True)
                    res = smpool.tile([1, HID], F32, tag=tag + "r", name=f"res_{tag}")
                    nc.vector.tensor_copy(out=res[:], in_=ps[:])
                    return res

                sw = []
                for r in range(2):
                    t = smpool.tile([128, HID], F32, tag=f"sw{r}")
                    nc.vector.tensor_scalar(out=t[:], in0=w_f32[r][:],
                                            scalar1=g_sb[r][:, C:C + 1], scalar2=None,
                                            op0=mybir.AluOpType.mult)
                    sw.append(t)
                mu_acc = psum_colsum(sw, "mua")
                wb = []
                for r in range(2):
                    t = smpool.tile([128, HID], F32, tag=f"wb{r}")
                    nc.vector.tensor_tensor(out=t[:], in0=w_f32[r][:], in1=b_sb[r][:],
                                            op=mybir.AluOpType.mult)
                    wb.append(t)
                d_acc = psum_colsum(wb, "da")
                invn = 1.0 / p.N
                mu = smpool.tile([1, HID], F32, tag="mu")
                nc.vector.tensor_scalar(out=mu[:], in0=mu_acc[:], scalar1=invn,
                                        scalar2=None, op0=mybir.AluOpType.mult)
                var = smpool.tile([1, HID], F32, tag="var")
                nc.vector.tensor_scalar(out=var[:], in0=d_acc[:], scalar1=invn,
                                        scalar2=None, op0=mybir.AluOpType.mult)
                musq = smpool.tile([1, HID], F32, tag="musq")
                nc.vector.tensor_tensor(out=musq[:], in0=mu[:], in1=mu[:],
                                        op=mybir.AluOpType.mult)
                nc.vector.tensor_tensor(out=var[:], in0=var[:], in1=musq[:],
                                        op=mybir.AluOpType.subtract)
                nc.vector.tensor_scalar(out=var[:], in0=var[:], scalar1=BN_EPS,
                                        scalar2=None, op0=mybir.AluOpType.add)
                sq = smpool.tile([1, HID], F32, tag="sq")
                nc.scalar.activation(out=sq[:], in_=var[:],
                                     func=mybir.ActivationFunctionType.Sqrt,
                                     bias=0.0, scale=1.0)
                rs = smpool.tile([1, HID], F32, tag="rs")
                nc.vector.reciprocal(out=rs[:], in_=sq[:])
                a_vec = smpool.tile([1, HID], F32, tag="av")
                nc.vector.tensor_tensor(out=a_vec[:], in0=rs[:], in1=gamma_sb[:],
                                        op=mybir.AluOpType.mult)
                b_vec = smpool.tile([1, HID], F32, tag="bv")
                nc.vector.tensor_tensor(out=b_vec[:], in0=mu[:], in1=a_vec[:],
                                        op=mybir.AluOpType.mult)
                nc.vector.tensor_tensor(out=b_vec[:], in0=beta_sb[:], in1=b_vec[:],
                                        op=mybir.AluOpType.subtract)
                ab_cat = smpool.tile([1, 2 * HID], F32, tag="abc")
                nc.vector.tensor_copy(out=ab_cat[:, :HID], in_=a_vec[:])
                nc.vector.tensor_copy(out=ab_cat[:, HID:], in_=b_vec[:])
                ps_ab = psA.tile([128, 2 * HID], F32, tag="zab", name="psab")
                nc.tensor.matmul(ps_ab[:], lhsT=ones1_sb[:], rhs=ab_cat[:],
                                 start=True, stop=True)
                b_row = cpool.tile([1, HID], BF16, tag="brow")
                nc.vector.tensor_copy(out=b_row[:], in_=b_vec[:])
                ones1_bf = cpool.tile([1, 128], BF16, tag="on16")
                nc.vector.tensor_copy(out=ones1_bf[:], in_=ones1_sb[:])
                wsc = []
                for r in range(2):
                    w = cpool.tile([128, HID], BF16, tag=f"wsc{r}")
                    nc.vector.tensor_tensor(out=w[:], in0=w_f32[r][:],
                                            in1=ps_ab[:, :HID],
                                            op=mybir.AluOpType.mult)
                    wsc.append(w)

                # ---- phase B: full h table (node-linear layout) ----
                SPAN = 28
                HSPAN = 14
                nspans = math.ceil(PCOLS / SPAN)
                for s in range(nspans):
                    j0 = s * SPAN
                    j1 = min(PCOLS, j0 + SPAN)
                    w_nodes = (j1 - j0) * 128
                    spans = []
                    for r in range(2):
                        sp = spool.tile([128, SPAN * 128], BF16, tag=f"sp{r}")
                        nc.sync.dma_start(
                            out=sp[:, :w_nodes],
                            in_=t_s1T[r * 128:(r + 1) * 128,
                                      j0 * 128:j0 * 128 + w_nodes])
                        spans.append(sp)
                    for j in range(j0, j1):
                        zc = psZ.tile([128, HID], F32, tag="zb")
                        coff = (j - j0) * 128
                        nc.tensor.matmul(zc[:], lhsT=spans[0][:, coff:coff + 128],
                                         rhs=wsc[0][:], start=True, stop=False)
                        nc.tensor.matmul(zc[:], lhsT=spans[1][:, coff:coff + 128],
                                         rhs=wsc[1][:], start=False, stop=False)
                        nc.tensor.matmul(zc[:], lhsT=ones1_bf[:], rhs=b_row[:],
                                         start=False, stop=True)
                        hs_i = j // HSPAN
                        ho = j % HSPAN
                        he = min(PCOLS, (hs_i + 1) * HSPAN) - hs_i * HSPAN
                        if ho == 0:
                            hsp = hpool.tile([128, HSPAN * HID], BF16, tag="hsp",
                                             name=f"hsp{hs_i % 3}")
                            phaseAB.hsp = hsp
                        hsp = phaseAB.hsp
                        if j % 2 == 0:
                            nc.vector.tensor_scalar(
                                out=hsp[:, ho * HID:(ho + 1) * HID], in0=zc[:],
                                scalar1=0.0, scalar2=None, op0=mybir.AluOpType.max)
                        else:
                            nc.scalar.activation(
                                out=hsp[:, ho * HID:(ho + 1) * HID], in_=zc[:],
                                func=mybir.ActivationFunctionType.Relu,
                                bias=0.0, scale=1.0)
                        if ho == he - 1:
                            nc.sync.dma_start(
                                out=t_h[:, hs_i * HSPAN * HID:(hs_i * HSPAN + he) * HID],
                                in_=hsp[:, :he * HID])
                # zero col (gather padding target)
                zz = hpool.tile([128, HID], BF16, tag="zz")
                nc.vector.memset(zz[:], 0.0)
                nc.sync.dma_start(out=t_h[:, PCOLS * HID:], in_=zz[:])

            if nloop > 1:
                with tc.For_i(0, nloop, 1):
                    phaseAB()
            else:
                phaseAB()

        # ---------------- phase C ----------------
        with (tc.tile_pool(name="c2", bufs=1) as cpool,
              tc.tile_pool(name="gat", bufs=1) as gpool,
              tc.tile_pool(name="epi", bufs=3) as epool,
              tc.tile_pool(name="psG", bufs=4, space="PSUM") as psG,
              tc.tile_pool(name="psT", bufs=2, space="PSUM") as psT,
              tc.tile_pool(name="psO", bufs=2, space="PSUM") as psO):
            idx_sb = cpool.tile([128, totcols], I16, tag="idx")
            nc.sync.dma_start(out=idx_sb[:], in_=t_idx[:])
            x0_sb = cpool.tile([128, tiles * HID], BF16, tag="x0all")
            nc.sync.dma_start(out=x0_sb[:], in_=t_x0[:])
            idB_sb = cpool.tile([128, 128], BF16, tag="idb")
            nc.sync.dma_start(out=idB_sb[:], in_=t_idB[:])
            ident_sb = cpool.tile([128, 128], F32, tag="idn")
            nc.sync.dma_start(out=ident_sb[:], in_=t_ident[:])
            wop2 = []
            for r in range(2):
                w = cpool.tile([128, HID], F32, tag=f"wo2{r}")
                nc.sync.dma_start(out=w[:], in_=t_wop[r * 128:(r + 1) * 128, :])
                wb = cpool.tile([128, HID], BF16, tag=f"wo2b{r}")
                nc.vector.tensor_scalar(out=wb[:], in0=w[:], scalar1=W_OP_SCALE,
                                        scalar2=None, op0=mybir.AluOpType.mult)
                wop2.append(wb)

            lo_ap = t_h[0:64, :].rearrange("p (c d) -> (p c) d", d=HID)
            hi_ap = t_h[64:128, :].rearrange("p (c d) -> (p c) d", d=HID)

            def phaseC():
                qn = [0]
                colpos = [0]
                gtiles = {}

                def gather_call(n_chunks, ap, pfx):
                    if n_chunks == 0:
                        return None
                    g = gpool.tile([128, n_chunks * HID], BF16,
                                   tag="g", name=f"g{pfx}{qn[0] % 8}")
                    o = colpos[0]
                    nc.gpsimd.dma_gather(
                        out_ap=g[:].rearrange("p (c d) -> p c d", d=HID),
                        in_ap=ap,
                        idxs_ap=idx_sb[:, o:o + n_chunks * 8],
                        num_idxs=n_chunks * 128, num_idxs_reg=n_chunks * 128,
                        elem_size=HID, single_packet=False,
                        queue_num=qn[0] % 4)
                    qn[0] += 1
                    colpos[0] += n_chunks * 8
                    return g

                for k in range(nsc):
                    glo = gather_call(sc_lo[k], lo_ap, "l")
                    ghi = gather_call(sc_hi[k], hi_ap, "h")
                    gtiles[k] = (glo, ghi)
                    # process tiles of this supercall
                    for t in range(k * SG, min(tiles, (k + 1) * SG)):
                        klo, khi = int(K_lo[t]), int(K_hi[t])
                        o_lo = int(lo_off[t] - lo_off[k * SG])
                        o_hi = int(hi_off[t] - hi_off[k * SG])
                        agg = psG.tile([128, HID], F32, tag="agg")
                        nmm = klo + khi
                        if nmm == 0:
                            nc.vector.memset(agg[:], 0.0)
                        ci = 0
                        for c in range(klo):
                            nc.tensor.matmul(
                                agg[:], lhsT=idB_sb[:],
                                rhs=glo[:, (o_lo + c) * HID:(o_lo + c + 1) * HID],
                                start=(ci == 0), stop=(ci == nmm - 1))
                            ci += 1
                        for c in range(khi):
                            nc.tensor.matmul(
                                agg[:], lhsT=idB_sb[:],
                                rhs=ghi[:, (o_hi + c) * HID:(o_hi + c + 1) * HID],
                                start=(ci == 0), stop=(ci == nmm - 1))
                            ci += 1
                        # epilogue
                        ESPAN = 7
                        es = t // ESPAN
                        eo = t % ESPAN
                        e0 = es * ESPAN
                        e1 = min(tiles, e0 + ESPAN)
                        if eo == 0:
                            outsp = epool.tile([128, ESPAN * HID], F32, tag="outsp",
                                               name=f"outsp{es % 2}")
                            phaseC.outsp = outsp
                        outsp = phaseC.outsp
                        sup = epool.tile([128, HID], F32, tag="sup")
                        x0sc = epool.tile([128, HID], F32, tag="x0sc")
                        nc.any.tensor_scalar(out=x0sc[:],
                                             in0=x0_sb[:, t * HID:(t + 1) * HID],
                                             scalar1=ALPHA / (1.0 - ALPHA), scalar2=None,
                                             op0=mybir.AluOpType.mult)
                        nc.vector.tensor_tensor(out=sup[:], in0=agg[:], in1=x0sc[:],
                                                op=mybir.AluOpType.add)
                        trp = psT.tile([128, HID], F32, tag="tr")
                        for r in range(2):
                            nc.tensor.transpose(out=trp[:, r * 128:(r + 1) * 128],
                                                in_=sup[:, r * 128:(r + 1) * 128],
                                                identity=ident_sb[:])
                        supT = epool.tile([128, HID], BF16, tag="supT")
                        nc.any.tensor_copy(out=supT[:], in_=trp[:])
                        ops = psO.tile([128, HID], F32, tag="o")
                        nc.tensor.matmul(ops[:], lhsT=supT[:, :128], rhs=wop2[0][:],
                                         start=True, stop=False)
                        nc.tensor.matmul(ops[:], lhsT=supT[:, 128:], rhs=wop2[1][:],
                                         start=False, stop=True)
                        ut = epool.tile([128, HID], F32, tag="u")
                        nc.vector.tensor_tensor(out=ut[:], in0=sup[:], in1=ops[:],
                                                op=mybir.AluOpType.add)
                        nc.scalar.activation(out=outsp[:, eo * HID:(eo + 1) * HID],
                                             in_=ut[:],
                                             func=mybir.ActivationFunctionType.Relu,
                                             bias=0.0, scale=OUT_SCALE * (1.0 - ALPHA))
                        if t == e1 - 1:
                            nc.sync.dma_start(out=t_out[:, e0 * HID:e1 * HID],
                                              in_=outsp[:, :(e1 - e0) * HID])

            if nloop > 1:
                with tc.For_i(0, nloop, 1):
                    phaseC()
            else:
                phaseC()

    nc.compile()
    return nc


def make_weight_inputs(prob, W_pre, gamma, beta_bn, W_op):
    return dict(
        wpre=np.asarray(W_pre, np.float32),
        gamma=np.asarray(gamma, np.float32).reshape(1, -1),
        beta=np.asarray(beta_bn, np.float32).reshape(1, -1),
        wop=np.asarray(W_op, np.float32),
    )


def unpack_out(prob, arr):
    """[128, tiles*HID] -> [tiles, 128, HID] (lane-major rows per tile)."""
    return arr.reshape(128, prob.tiles, prob.HID).transpose(1, 0, 2)


# ======================================================================
# Self-contained execution via PJRT (axon) and public kernel() entry
# ======================================================================
import jax
from jax.sharding import Mesh, PartitionSpec, NamedSharding
from jax.experimental.shard_map import shard_map
from concourse.bass2jax import _bass_exec_p, install_neuronx_cc_hook, partition_id_tensor


def _build_exec(nc, n_cores):
    install_neuronx_cc_hook()
    partition_name = nc.partition_id_tensor.name if nc.partition_id_tensor else None
    in_names, out_names, out_avals, zero_outs = [], [], [], []
    for alloc in nc.m.functions[0].allocations:
        if not isinstance(alloc, mybir.MemoryLocationSet):
            continue
        name = alloc.memorylocations[0].name
        if alloc.kind == "ExternalInput":
            if name != partition_name:
                in_names.append(name)
        elif alloc.kind == "ExternalOutput":
            shape = tuple(alloc.tensor_shape)
            dtype = mybir.dt.np(alloc.dtype)
            out_names.append(name)
            out_avals.append(jax.core.ShapedArray(shape, dtype))
            zero_outs.append(np.zeros(shape, dtype))
    n_params = len(in_names)
    n_outs = len(out_avals)
    all_in_names = list(in_names) + list(out_names)
    if partition_name is not None:
        all_in_names.append(partition_name)

    def _body(*args):
        operands = list(args)
        if partition_name is not None:
            operands.append(partition_id_tensor())
        outs = _bass_exec_p.bind(
            *operands, out_avals=tuple(out_avals), in_names=tuple(all_in_names),
            out_names=tuple(out_names), lowering_input_output_aliases=(),
            sim_require_finite=True, sim_require_nnan=True, nc=nc)
        return tuple(outs)

    devices = jax.devices()[:n_cores]
    mesh = Mesh(np.asarray(devices), ("core",))
    in_specs = (PartitionSpec("core"),) * (n_params + n_outs)
    out_specs = (PartitionSpec("core"),) * n_outs
    donate = tuple(range(n_params, n_params + n_outs))
    fn = jax.jit(shard_map(_body, mesh=mesh, in_specs=in_specs,
                           out_specs=out_specs, check_rep=False),
                 donate_argnums=donate, keep_unused=True)
    return dict(fn=fn, in_names=in_names, out_names=out_names,
                out_avals=out_avals, zero_outs=zero_outs, mesh=mesh,
                n_cores=n_cores)


def _place_inputs(ex, in_maps):
    sh = NamedSharding(ex["mesh"], PartitionSpec("core"))
    n_cores = ex["n_cores"]
    return [jax.device_put(
        np.concatenate([np.asarray(in_maps[c][name]) for c in range(n_cores)], axis=0), sh)
        for name in ex["in_names"]]


def _run(ex, dev_in):
    sh = NamedSharding(ex["mesh"], PartitionSpec("core"))
    n_cores = ex["n_cores"]
    zs = [jax.device_put(np.zeros((n_cores * z.shape[0], *z.shape[1:]), z.dtype), sh)
          for z in ex["zero_outs"]]
    outs = jax.block_until_ready(ex["fn"](*dev_in, *zs))
    return [
        {name: np.asarray(outs[i]).reshape(n_cores, *ex["out_avals"][i].shape)[c]
         for i, name in enumerate(ex["out_names"])}
        for c in range(n_cores)
    ]


_CACHE = {}


def _get_compiled(prob, meta, W_pre, gamma, beta_bn, W_op, key):
    if key not in _CACHE:
        nc = build_kernel(prob, meta, W_pre, gamma, beta_bn, W_op, nloop=1)
        _CACHE[key] = _build_exec(nc, prob.n_cores)
    return _CACHE[key]


def kernel(s0=None, s1=None, x_0=None, W_pre=None, gamma=None, beta_bn=None,
           W_op=None, edge_index=None, drop_prob=None, training=None, **_ignored):
    s1 = np.asarray(s1, np.float32)
    x_0 = np.asarray(x_0, np.float32)
    W_pre = np.asarray(W_pre, np.float32)
    gamma = np.asarray(gamma, np.float32)
    beta_bn = np.asarray(beta_bn, np.float32)
    W_op = np.asarray(W_op, np.float32)
    edge_index = np.asarray(edge_index)
    N, C = s1.shape
    HID = W_pre.shape[1]
    E = edge_index.shape[1]
    prob = Prob(N, E, C, HID, n_cores=8)
    in_maps, meta = host_prep(prob, s1, x_0, edge_index)
    key = (N, E, C, HID, int(np.int64(edge_index[:, ::97]).sum()), meta["totcols"])
    ex = _get_compiled(prob, meta, W_pre, gamma, beta_bn, W_op, key)
    wins = make_weight_inputs(prob, W_pre, gamma, beta_bn, W_op)
    full_maps = [{**m, **wins} for m in in_maps]
    dev_in = _place_inputs(ex, full_maps)
    res = _run(ex, dev_in)
    out = np.zeros((N, HID), np.float32)
    out_map = meta["out_map"]
    for m in range(prob.n_cores):
        rows = unpack_out(prob, res[m]["out"])          # [tiles, 128, HID]
        nd = out_map[m]                                 # [tiles, 128]
        ok = nd >= 0
        out[nd[ok]] = rows[ok]
    return np.ascontiguousarray(out).astype(np.float32)


# revision 9
# speedup vs baseline: 1.7992x; 1.7992x over previous
"""GNN message-passing kernel for trn2: h = relu(BN(s1 @ W_pre));
agg = segment_sum(h[src], dst); out = relu((1-b)*support + b*support@W_op),
support = 0.9*(h+agg) + 0.1*x_0.

Sharding: phase 1 (h) replicated on all 8 cores; phase 2 (aggregate+output)
sharded by destination node. Gather via dma_gather on bf16 h tables (lo/hi
split for int16 indices). Segment-sum via selection-matrix matmuls.
"""
import math
import numpy as np
import ml_dtypes

import concourse.bass as bass
import concourse.bacc as bacc
import concourse.mybir as mybir
from concourse.tile import TileContext

BF16 = mybir.dt.bfloat16
F32 = mybir.dt.float32
I16 = mybir.dt.int16

ALPHA = 0.1
LAMBDA = 0.5
BN_EPS = 1e-5
BETA_C = float(np.log(LAMBDA / 1.0 + 1.0))   # 0.405465
W_OP_SCALE = BETA_C / (1.0 - BETA_C)         # fold: u = support + support@ (W_op*W_OP_SCALE)
OUT_SCALE = 1.0 - BETA_C                     # out = relu(OUT_SCALE * u)


class Prob:
    def __init__(self, N, E, C, HID, n_cores):
        self.N, self.E, self.C, self.HID, self.n_cores = N, E, C, HID, n_cores
        assert C == 256 and HID == 256
        self.shard = N // n_cores                      # dst nodes per core (must divide)
        assert self.shard * n_cores == N
        self.tiles = math.ceil(self.shard / 128)       # dst tiles per core
        self.shard_pad = self.tiles * 128
        self.nchunks = math.ceil(N / 128)              # node chunks for h
        self.npad = self.nchunks * 128
        self.lo_chunks = (self.nchunks + 1) // 2       # h_lo = chunks [0, lo_chunks)
        self.V_lo = self.lo_chunks * 128
        self.V_hi = self.npad - self.V_lo
        assert self.V_lo < 32768 and self.V_hi < 32768


def host_prep(prob, s1, x_0, edge_index):
    """Build per-core input maps + layout metadata. All numpy."""
    p = prob
    N, E, M = p.N, p.E, p.n_cores
    s1 = np.asarray(s1, dtype=np.float32)
    x_0 = np.asarray(x_0, dtype=np.float32)
    src = np.asarray(edge_index[0], dtype=np.int64)
    dst = np.asarray(edge_index[1], dtype=np.int64)
    # append self edges i->i (the GIN +h term)
    src = np.concatenate([src, np.arange(N, dtype=np.int64)])
    dst = np.concatenate([dst, np.arange(N, dtype=np.int64)])

    core = dst // p.shard
    rel = dst - core * p.shard
    trel = rel >> 7
    prel = rel & 127
    half = (src >= p.V_lo).astype(np.int64)
    gid = (core * p.tiles + trel) * 2 + half
    ngroups = M * p.tiles * 2
    order = np.argsort(gid, kind="stable")
    # counts per group
    cnt = np.bincount(gid, minlength=ngroups).reshape(M, p.tiles, 2)
    # chunks per (tile, half): max over cores (shared NEFF layout)
    K = np.maximum(np.ceil(cnt / 128.0).astype(np.int64).max(axis=0), 1)  # [tiles, 2]
    slots = K * 128
    # column offsets of each (t, half) group in the concatenated layout
    off = np.zeros((p.tiles, 2), np.int64)
    run = 0
    for t in range(p.tiles):
        for h in (0, 1):
            off[t, h] = run
            run += K[t, h]
    ktot = run                       # total chunks per core
    # build per-core padded idx/drel arrays
    src_s = src[order]
    drel_s = prel[order]
    gid_s = gid[order]
    # starts of each group in sorted arrays
    gstart = np.zeros(ngroups + 1, np.int64)
    np.cumsum(np.bincount(gid_s, minlength=ngroups), out=gstart[1:])
    idx_flat = np.zeros((M, ktot * 128), np.int16)
    drel_flat = np.full((M, ktot * 128), 200.0, np.float32)  # cast later
    for m in range(M):
        for t in range(p.tiles):
            for h in (0, 1):
                g = (m * p.tiles + t) * 2 + h
                a, b = gstart[g], gstart[g + 1]
                n = b - a
                base = off[t, h] * 128
                v = src_s[a:b] - (p.V_lo if h else 0)
                # chunk-major h table: node (j*128+p') lives at flat row p'*tchunks + j
                tch = p.lo_chunks if h == 0 else (p.nchunks - p.lo_chunks)
                v = (v & 127) * tch + (v >> 7)
                idx_flat[m, base:base + n] = v.astype(np.int16)
                drel_flat[m, base:base + n] = drel_s[a:b].astype(np.float32)
    # dma_gather idx layout: index j at [j%16, j//16], replicated x8 down partitions
    idx_lay = idx_flat.reshape(M, ktot * 8, 16).transpose(0, 2, 1)  # [M, 16, ktot*8]
    idx_lay = np.tile(idx_lay, (1, 8, 1))                           # [M, 128, ktot*8]
    # drel layout: [128, ktot]: drel[p, c] = flat[c*128 + p]
    drel_lay = np.ascontiguousarray(drel_flat.reshape(M, ktot, 128).transpose(0, 2, 1))

    # s1 extended, padded to 392 chunks so the Gram shards evenly (49/core)
    nch2 = 392
    s1e = np.zeros((nch2 * 128, p.C + 1), np.float32)
    s1e[:N, :p.C] = s1
    s1e[:N, p.C] = 1.0
    s1e = s1e.reshape(nch2, 128, p.C + 1).transpose(1, 0, 2)   # [128, ch, 257]
    s1e = np.ascontiguousarray(s1e).astype(ml_dtypes.bfloat16)
    gsh = nch2 // M
    s1e_sh = [np.ascontiguousarray(s1e[:, m * gsh:(m + 1) * gsh, :]).reshape(128, -1)
              for m in range(M)]
    # s1 transposed [C, npad] bf16
    s1T = np.zeros((p.C, p.npad), np.float32)
    s1T[:, :N] = s1.T
    s1T = np.ascontiguousarray(s1T).astype(ml_dtypes.bfloat16)

    x0s = np.zeros((M, p.shard_pad, p.HID), np.float32)
    for m in range(M):
        x0s[m, :p.shard] = x_0[m * p.shard:(m + 1) * p.shard]
    x0s = x0s.reshape(M, p.tiles, 128, p.HID).transpose(0, 2, 1, 3).reshape(M, 128, -1)
    x0s = np.ascontiguousarray(x0s)

    iota = np.broadcast_to(np.arange(128, dtype=np.float32), (128, 128)).astype(ml_dtypes.bfloat16).copy()
    ident = np.eye(128, dtype=np.float32)
    ones1 = np.ones((1, 128), np.float32)
    onesc = np.ones((128, 1), np.float32)

    meta = dict(K=K, off=off, ktot=ktot)
    shared = dict(s1T=np.asarray(s1T), iota=iota, ident=ident,
                  ones1=ones1, onesc=onesc)
    in_maps = []
    for m in range(M):
        d = dict(shared)
        d["s1e"] = s1e_sh[m]
        d["idxall"] = idx_lay[m]
        d["drel"] = drel_lay[m]
        d["x0s"] = x0s[m]
        in_maps.append(d)
    return in_maps, meta


def build_kernel(prob, meta, W_pre, gamma, beta_bn, W_op, nloop=1, nq=4, phases='ABC', c_parts='gse'):
    """Build + compile the Bacc kernel. Weights are compile-time-ish inputs
    (still passed as tensors; only meta layout is baked)."""
    p = prob
    K, off, ktot = meta["K"], meta["off"], meta["ktot"]
    C, HID = p.C, p.HID
    nc = bacc.Bacc("TRN2", target_bir_lowering=False, debug=False,
                   num_devices=p.n_cores, num_swdge_queues=nq)
    GSH = 49
    t_s1e = nc.dram_tensor("s1e", [128, GSH * (C + 1)], BF16, kind="ExternalInput")
    cc_in = nc.dram_tensor("cc_in", [256, C + 1], F32)
    cc_out = nc.dram_tensor("cc_out", [256, C + 1], F32, addr_space="Shared")
    t_s1T = nc.dram_tensor("s1T", [C, p.npad], BF16, kind="ExternalInput")
    t_wpre = nc.dram_tensor("wpre", [C, HID], F32, kind="ExternalInput")
    t_gamma = nc.dram_tensor("gamma", [1, HID], F32, kind="ExternalInput")
    t_beta = nc.dram_tensor("beta", [1, HID], F32, kind="ExternalInput")
    t_wop = nc.dram_tensor("wop", [HID, HID], F32, kind="ExternalInput")
    t_x0 = nc.dram_tensor("x0s", [128, p.tiles * HID], F32, kind="ExternalInput")
    t_idx = nc.dram_tensor("idxall", [128, ktot * 8], I16, kind="ExternalInput")
    t_drel = nc.dram_tensor("drel", [128, ktot], F32, kind="ExternalInput")
    t_iota = nc.dram_tensor("iota", [128, 128], BF16, kind="ExternalInput")
    t_ident = nc.dram_tensor("ident", [128, 128], F32, kind="ExternalInput")
    t_ones1 = nc.dram_tensor("ones1", [1, 128], F32, kind="ExternalInput")
    t_onesc = nc.dram_tensor("onesc", [128, 1], F32, kind="ExternalInput")
    t_out = nc.dram_tensor("out", [128, p.tiles * HID], F32, kind="ExternalOutput")
    lo_ch = p.lo_chunks
    hi_ch = p.nchunks - p.lo_chunks
    h_lo = nc.dram_tensor("h_lo", [128, lo_ch * HID], BF16)
    h_hi = nc.dram_tensor("h_hi", [128, hi_ch * HID], BF16)

    SPAN = 64  # chunks per s1T span load

    # ---------------- context 1: stats + h ----------------
    tc1 = TileContext(nc)
    with tc1 as tc:
        with (tc.tile_pool(name="const", bufs=1) as cpool,
              tc.tile_pool(name="s1in", bufs=4) as apool,
              tc.tile_pool(name="span", bufs=2) as spool,
              tc.tile_pool(name="hout", bufs=3) as hpool,
              tc.tile_pool(name="psA", bufs=1, space="PSUM") as psA,
              tc.tile_pool(name="psZ", bufs=5, space="PSUM") as psZ,
              tc.tile_pool(name="small", bufs=1) as smpool):
            # constants
            w_f32 = []
            for r in range(2):
                w = cpool.tile([128, HID], F32, tag=f"wf{r}")
                nc.sync.dma_start(out=w[:], in_=t_wpre[r * 128:(r + 1) * 128, :])
                w_f32.append(w)
            gamma_sb = cpool.tile([1, HID], F32, tag="gm")
            nc.sync.dma_start(out=gamma_sb[:], in_=t_gamma[:])
            beta_sb = cpool.tile([1, HID], F32, tag="bt")
            nc.sync.dma_start(out=beta_sb[:], in_=t_beta[:])
            ones1_sb = cpool.tile([1, 128], F32, tag="on")
            nc.sync.dma_start(out=ones1_sb[:], in_=t_ones1[:])
            onesc_sb = cpool.tile([128, 1], F32, tag="onc")
            nc.sync.dma_start(out=onesc_sb[:], in_=t_onesc[:])
            wop_f32 = []
            for r in range(2):
                w = cpool.tile([128, HID], F32, tag=f"wo{r}")
                nc.sync.dma_start(out=w[:], in_=t_wop[r * 128:(r + 1) * 128, :])
                wop_f32.append(w)

            # ---- phase A: Gram sharded 8-way + AllReduce ----
            gps = [psA.tile([128, C + 1], F32, tag=f"g{r}", name=f"gps{r}") for r in range(2)]
            ASPAN = 25
            naspans = math.ceil(GSH / ASPAN)
            CW = C + 1
            def phaseA():
                for s in range(naspans):
                    j0 = s * ASPAN
                    j1 = min(GSH, j0 + ASPAN)
                    s1t = apool.tile([128, ASPAN * CW], BF16, name="s1span")
                    nc.sync.dma_start(out=s1t[:, :(j1 - j0) * CW],
                                      in_=t_s1e[:, j0 * CW:j1 * CW])
                    for j in range(j0, j1):
                        co = (j - j0) * CW
                        for r in range(2):
                            nc.tensor.matmul(gps[r][:],
                                             lhsT=s1t[:, co + r * 128:co + r * 128 + 128],
                                             rhs=s1t[:, co:co + CW], start=(j == 0),
                                             stop=(j == GSH - 1))
            phaseA()

            # ---- AllReduce partial grams; stats finalize ----
            for r in range(2):
                g = smpool.tile([128, C + 1], F32, tag=f"gp{r}")
                nc.vector.tensor_copy(out=g[:], in_=gps[r][:])
                nc.sync.dma_start(out=cc_in[r * 128:(r + 1) * 128, :], in_=g[:])
            nc.gpsimd.collective_compute(
                "AllReduce", mybir.AluOpType.add,
                replica_groups=[list(range(p.n_cores))],
                ins=[cc_in[:]], outs=[cc_out[:]])
            g_sb = []
            for r in range(2):
                g = smpool.tile([128, C + 1], F32, tag=f"gsb{r}")
                nc.sync.dma_start(out=g[:], in_=cc_out[r * 128:(r + 1) * 128, :])
                g_sb.append(g)
            # B_mat = G @ W  (G symmetric; lhsT = G rows as [K,M])
            psB = psZ.tile([128, HID], F32, tag="zb")
            b_sb = []
            for r in range(2):
                for k in range(2):
                    nc.tensor.matmul(psB[:], lhsT=g_sb[k][:, r * 128:(r + 1) * 128],
                                     rhs=w_f32[k][:], start=(k == 0), stop=(k == 1))
                b = smpool.tile([128, HID], F32, tag=f"bsb{r}")
                nc.vector.tensor_copy(out=b[:], in_=psB[:])
                b_sb.append(b)

            def psum_colsum(tiles_in, tag):
                # sum over partitions of (tiles_in[0]+tiles_in[1]) -> [1, HID] in SBUF
                acc = smpool.tile([128, HID], F32, tag=tag)
                nc.vector.tensor_tensor(out=acc[:], in0=tiles_in[0][:],
                                        in1=tiles_in[1][:], op=mybir.AluOpType.add)
                ps = psA.tile([1, HID], F32, tag="zred", name=f"ps_{tag}")
                nc.tensor.matmul(ps[:], lhsT=onesc_sb[:], rhs=acc[:],
                                 start=True, stop=True)
                res = smpool.tile([1, HID], F32, tag=tag + "r", name=f"res_{tag}")
                nc.vector.tensor_copy(out=res[:], in_=ps[:])
                return res

            # mu_raw = sum_k s_k W[k, :]
            sw = []
            for r in range(2):
                t = smpool.tile([128, HID], F32, tag=f"sw{r}")
                nc.vector.tensor_scalar(out=t[:], in0=w_f32[r][:],
                                        scalar1=g_sb[r][:, C:C + 1], scalar2=None,
                                        op0=mybir.AluOpType.mult)
                sw.append(t)
            mu_acc = psum_colsum(sw, "mua")
            # d_raw = sum_k W[k,c] B[k,c]
            wb = []
            for r in range(2):
                t = smpool.tile([128, HID], F32, tag=f"wb{r}")
                nc.vector.tensor_tensor(out=t[:], in0=w_f32[r][:], in1=b_sb[r][:],
                                        op=mybir.AluOpType.mult)
                wb.append(t)
            d_acc = psum_colsum(wb, "da")
            invn = 1.0 / p.N
            mu = smpool.tile([1, HID], F32, tag="mu")
            nc.vector.tensor_scalar(out=mu[:], in0=mu_acc[:], scalar1=invn,
                                    scalar2=None, op0=mybir.AluOpType.mult)
            var = smpool.tile([1, HID], F32, tag="var")
            # var = d/N - mu^2
            nc.vector.tensor_scalar(out=var[:], in0=d_acc[:], scalar1=invn,
                                    scalar2=None, op0=mybir.AluOpType.mult)
            musq = smpool.tile([1, HID], F32, tag="musq")
            nc.vector.tensor_tensor(out=musq[:], in0=mu[:], in1=mu[:],
                                    op=mybir.AluOpType.mult)
            nc.vector.tensor_tensor(out=var[:], in0=var[:], in1=musq[:],
                                    op=mybir.AluOpType.subtract)
            nc.vector.tensor_scalar(out=var[:], in0=var[:], scalar1=BN_EPS,
                                    scalar2=None, op0=mybir.AluOpType.add)
            sq = smpool.tile([1, HID], F32, tag="sq")
            nc.scalar.activation(out=sq[:], in_=var[:],
                                 func=mybir.ActivationFunctionType.Sqrt,
                                 bias=0.0, scale=1.0)
            rs = smpool.tile([1, HID], F32, tag="rs")
            nc.vector.reciprocal(out=rs[:], in_=sq[:])
            a_vec = smpool.tile([1, HID], F32, tag="av")
            nc.vector.tensor_tensor(out=a_vec[:], in0=rs[:], in1=gamma_sb[:],
                                    op=mybir.AluOpType.mult)
            b_vec = smpool.tile([1, HID], F32, tag="bv")
            nc.vector.tensor_tensor(out=b_vec[:], in0=mu[:], in1=a_vec[:],
                                    op=mybir.AluOpType.mult)
            nc.vector.tensor_tensor(out=b_vec[:], in0=beta_sb[:], in1=b_vec[:],
                                    op=mybir.AluOpType.subtract)
            # broadcast A|B to 128 partitions via K=1 matmul
            ab_cat = smpool.tile([1, 2 * HID], F32, tag="abc")
            nc.vector.tensor_copy(out=ab_cat[:, :HID], in_=a_vec[:])
            nc.vector.tensor_copy(out=ab_cat[:, HID:], in_=b_vec[:])
            ps_ab = psZ.tile([128, 2 * HID], F32, tag="zb")
            nc.tensor.matmul(ps_ab[:], lhsT=ones1_sb[:], rhs=ab_cat[:],
                             start=True, stop=True)
            b_bc = cpool.tile([128, HID], F32, tag="bbc")
            nc.vector.tensor_copy(out=b_bc[:], in_=ps_ab[:, HID:])
            # scaled weights: Wsc = W * A (bf16); B row for K=1 add; Wop scaled
            wsc = []
            for r in range(2):
                w = cpool.tile([128, HID], BF16, tag=f"wsc{r}")
                nc.vector.tensor_tensor(out=w[:], in0=w_f32[r][:], in1=ps_ab[:, :HID],
                                        op=mybir.AluOpType.mult)
                wsc.append(w)
            b_row = cpool.tile([1, HID], BF16, tag="brow")
            nc.vector.tensor_copy(out=b_row[:], in_=b_vec[:])
            ones1_bf = cpool.tile([1, 128], BF16, tag="on16")
            nc.vector.tensor_copy(out=ones1_bf[:], in_=ones1_sb[:])
            wopsc = []
            for r in range(2):
                w = cpool.tile([128, HID], BF16, tag=f"wosc{r}")
                nc.vector.tensor_scalar(out=w[:], in0=wop_f32[r][:],
                                        scalar1=W_OP_SCALE, scalar2=None,
                                        op0=mybir.AluOpType.mult)
                wopsc.append(w)

            # ---- phase B: z = s1 @ Wsc (+B) -> relu -> h ----
            nspans = math.ceil(p.nchunks / SPAN)
            def phaseB():
                for s in range(nspans):
                    j0 = s * SPAN
                    j1 = min(p.nchunks, j0 + SPAN)
                    w_nodes = (j1 - j0) * 128
                    spans = []
                    for r in range(2):
                        sp = spool.tile([128, SPAN * 128], BF16, tag=f"sp{r}")
                        nc.sync.dma_start(
                            out=sp[:, :w_nodes],
                            in_=t_s1T[r * 128:(r + 1) * 128, j0 * 128:j0 * 128 + w_nodes])
                        spans.append(sp)
                    for j in range(j0, j1):
                        zc = psZ.tile([128, HID], F32, tag="zb")
                        coff = (j - j0) * 128
                        nc.tensor.matmul(zc[:], lhsT=spans[0][:, coff:coff + 128],
                                         rhs=wsc[0][:], start=True, stop=False)
                        nc.tensor.matmul(zc[:], lhsT=spans[1][:, coff:coff + 128],
                                         rhs=wsc[1][:], start=False, stop=False)
                        nc.tensor.matmul(zc[:], lhsT=ones1_bf[:], rhs=b_row[:],
                                         start=False, stop=True)
                        # h span buffering (chunk-major tables, 14-chunk spans)
                        HSPAN = 14
                        if j < p.lo_chunks:
                            tbl, jj, nch = h_lo, j, lo_ch
                        else:
                            tbl, jj, nch = h_hi, j - p.lo_chunks, hi_ch
                        hs = jj // HSPAN
                        ho = jj % HSPAN
                        he = min(nch, (hs + 1) * HSPAN) - hs * HSPAN
                        if ho == 0:
                            hsp = hpool.tile([128, HSPAN * HID], BF16, tag="hsp",
                                             name=f"hsp_{0 if tbl is h_lo else 1}_{hs % 3}")
                            phaseB.hsp = hsp
                        hsp = phaseB.hsp
                        if j % 2 == 0:
                            nc.vector.tensor_scalar(
                                out=hsp[:, ho * HID:(ho + 1) * HID], in0=zc[:],
                                scalar1=0.0, scalar2=None, op0=mybir.AluOpType.max)
                        else:
                            nc.scalar.activation(
                                out=hsp[:, ho * HID:(ho + 1) * HID], in_=zc[:],
                                func=mybir.ActivationFunctionType.Relu,
                                bias=0.0, scale=1.0)
                        if ho == he - 1:
                            nc.sync.dma_start(
                                out=tbl[:, hs * HSPAN * HID:(hs * HSPAN + he) * HID],
                                in_=hsp[:, :he * HID])
            if 'B' in phases:
                if nloop > 1:
                    with tc.For_i(0, nloop, 1):
                        phaseB()
                else:
                    phaseB()
            else:
                phaseB()  # once

        # ---------------- context 2: aggregate + output ----------------
        with (tc.tile_pool(name="c2", bufs=1) as cpool,
              tc.tile_pool(name="gat", bufs=10) as gpool,
              tc.tile_pool(name="sel", bufs=8) as selp,
              tc.tile_pool(name="epi", bufs=3) as epool,
              tc.tile_pool(name="psG", bufs=4, space="PSUM") as psG,
              tc.tile_pool(name="psT", bufs=2, space="PSUM") as psT,
              tc.tile_pool(name="psO", bufs=2, space="PSUM") as psO):
            idx_sb = cpool.tile([128, ktot * 8], I16, tag="idx")
            nc.sync.dma_start(out=idx_sb[:], in_=t_idx[:])
            drel_sb = cpool.tile([128, ktot], F32, tag="dr")
            nc.sync.dma_start(out=drel_sb[:], in_=t_drel[:])
            iota_sb = cpool.tile([128, 128], BF16, tag="io")
            nc.sync.dma_start(out=iota_sb[:], in_=t_iota[:])
            ident_sb = cpool.tile([128, 128], F32, tag="idn")
            nc.sync.dma_start(out=ident_sb[:], in_=t_ident[:])
            wop2 = []
            for r in range(2):
                w = cpool.tile([128, HID], F32, tag=f"wo2{r}")
                nc.sync.dma_start(out=w[:], in_=t_wop[r * 128:(r + 1) * 128, :])
                wb = cpool.tile([128, HID], BF16, tag=f"wo2b{r}")
                nc.vector.tensor_scalar(out=wb[:], in0=w[:], scalar1=W_OP_SCALE,
                                        scalar2=None, op0=mybir.AluOpType.mult)
                wop2.append(wb)

            qn = [0]
            def phaseC():
                for t in range(p.tiles):
                    gt = {}
                    for hh in (0, 1) if 'g' in c_parts else ():
                        kk = int(K[t, hh])
                        g = gpool.tile([128, kk * HID], BF16, tag=f"g{hh}")
                        tbl = h_lo if hh == 0 else h_hi
                        o8 = int(off[t, hh]) * 8
                        nc.gpsimd.dma_gather(
                            out_ap=g[:].rearrange("p (c d) -> p c d", d=HID),
                            in_ap=tbl[:].rearrange("p (c d) -> (p c) d", d=HID),
                            idxs_ap=idx_sb[:, o8:o8 + kk * 8],
                            num_idxs=kk * 128, num_idxs_reg=kk * 128,
                            elem_size=HID, single_packet=False,
                            queue_num=qn[0] % 4)
                        qn[0] += 1
                        gt[hh] = g
                    agg = psG.tile([128, HID], F32, tag="agg")
                    nmm = int(K[t, 0] + K[t, 1])
                    ci = 0
                    if 's' not in c_parts or 'g' not in c_parts:
                        nc.vector.memset(agg[:], 0.0)
                    for hh in ((0, 1) if ('s' in c_parts and 'g' in c_parts) else ()):
                        kk = int(K[t, hh])
                        for c in range(kk):
                            col = int(off[t, hh]) + c
                            S = selp.tile([128, 128], BF16)
                            nc.vector.tensor_scalar(
                                out=S[:], in0=iota_sb[:],
                                scalar1=drel_sb[:, col:col + 1], scalar2=None,
                                op0=mybir.AluOpType.is_equal)
                            nc.tensor.matmul(agg[:], lhsT=S[:],
                                             rhs=gt[hh][:, c * HID:(c + 1) * HID],
                                             start=(ci == 0), stop=(ci == nmm - 1))
                            ci += 1
                    # epilogue
                    if 'e' not in c_parts:
                        continue
                    ESPAN = 7
                    es = t // ESPAN
                    eo = t % ESPAN
                    e0 = es * ESPAN
                    e1 = min(p.tiles, e0 + ESPAN)
                    if eo == 0:
                        x0sp = epool.tile([128, ESPAN * HID], F32, tag="x0sp",
                                          name=f"x0sp{es % 2}")
                        nc.sync.dma_start(out=x0sp[:, :(e1 - e0) * HID],
                                          in_=t_x0[:, e0 * HID:e1 * HID])
                        outsp = epool.tile([128, ESPAN * HID], F32, tag="outsp",
                                           name=f"outsp{es % 2}")
                        phaseC.x0sp, phaseC.outsp = x0sp, outsp
                    x0sp, outsp = phaseC.x0sp, phaseC.outsp
                    sup = epool.tile([128, HID], F32, tag="sup")
                    # sup = 0.9*agg + 0.1*x0
                    nc.vector.tensor_scalar(out=sup[:], in0=agg[:],
                                            scalar1=(1.0 - ALPHA), scalar2=None,
                                            op0=mybir.AluOpType.mult)
                    x0sc = epool.tile([128, HID], F32, tag="x0sc")
                    nc.any.tensor_scalar(out=x0sc[:], in0=x0sp[:, eo * HID:(eo + 1) * HID],
                                         scalar1=ALPHA, scalar2=None,
                                         op0=mybir.AluOpType.mult)
                    nc.vector.tensor_tensor(out=sup[:], in0=sup[:], in1=x0sc[:],
                                            op=mybir.AluOpType.add)
                    # transpose sup -> supT (bf16)
                    trp = psT.tile([128, HID], F32, tag="tr")
                    for r in range(2):
                        nc.tensor.transpose(out=trp[:, r * 128:(r + 1) * 128],
                                            in_=sup[:, r * 128:(r + 1) * 128],
                                            identity=ident_sb[:])
                    supT = epool.tile([128, HID], BF16, tag="supT")
                    nc.any.tensor_copy(out=supT[:], in_=trp[:])
                    ops = psO.tile([128, HID], F32, tag="o")
                    nc.tensor.matmul(ops[:], lhsT=supT[:, :128], rhs=wop2[0][:],
                                     start=True, stop=False)
                    nc.tensor.matmul(ops[:], lhsT=supT[:, 128:], rhs=wop2[1][:],
                                     start=False, stop=True)
                    ut = epool.tile([128, HID], F32, tag="u")
                    nc.vector.tensor_tensor(out=ut[:], in0=sup[:], in1=ops[:],
                                            op=mybir.AluOpType.add)
                    nc.scalar.activation(out=outsp[:, eo * HID:(eo + 1) * HID],
                                         in_=ut[:],
                                         func=mybir.ActivationFunctionType.Relu,
                                         bias=0.0, scale=OUT_SCALE)
                    if t == e1 - 1:
                        nc.sync.dma_start(out=t_out[:, e0 * HID:e1 * HID],
                                          in_=outsp[:, :(e1 - e0) * HID])
            if 'C' in phases:
                if nloop > 1:
                    with tc.For_i(0, nloop, 1):
                        phaseC()
                else:
                    phaseC()

    nc.compile()
    return nc


def make_weight_inputs(prob, W_pre, gamma, beta_bn, W_op):
    return dict(
        wpre=np.asarray(W_pre, np.float32),
        gamma=np.asarray(gamma, np.float32).reshape(1, -1),
        beta=np.asarray(beta_bn, np.float32).reshape(1, -1),
        wop=np.asarray(W_op, np.float32),
    )


def unpack_out(prob, arr):
    """[128, tiles*HID] chunk-major -> [shard_pad, HID]"""
    return arr.reshape(128, prob.tiles, prob.HID).transpose(1, 0, 2).reshape(
        prob.shard_pad, prob.HID)


# ======================================================================
# Self-contained execution via PJRT (axon) and public kernel() entry
# ======================================================================
import jax
from jax.sharding import Mesh, PartitionSpec, NamedSharding
from jax.experimental.shard_map import shard_map
from concourse.bass2jax import _bass_exec_p, install_neuronx_cc_hook, partition_id_tensor


def _build_exec(nc, n_cores):
    install_neuronx_cc_hook()
    partition_name = nc.partition_id_tensor.name if nc.partition_id_tensor else None
    in_names, out_names, out_avals, zero_outs = [], [], [], []
    for alloc in nc.m.functions[0].allocations:
        if not isinstance(alloc, mybir.MemoryLocationSet):
            continue
        name = alloc.memorylocations[0].name
        if alloc.kind == "ExternalInput":
            if name != partition_name:
                in_names.append(name)
        elif alloc.kind == "ExternalOutput":
            shape = tuple(alloc.tensor_shape)
            dtype = mybir.dt.np(alloc.dtype)
            out_names.append(name)
            out_avals.append(jax.core.ShapedArray(shape, dtype))
            zero_outs.append(np.zeros(shape, dtype))
    n_params = len(in_names)
    n_outs = len(out_avals)
    all_in_names = list(in_names) + list(out_names)
    if partition_name is not None:
        all_in_names.append(partition_name)

    def _body(*args):
        operands = list(args)
        if partition_name is not None:
            operands.append(partition_id_tensor())
        outs = _bass_exec_p.bind(
            *operands, out_avals=tuple(out_avals), in_names=tuple(all_in_names),
            out_names=tuple(out_names), lowering_input_output_aliases=(),
            sim_require_finite=True, sim_require_nnan=True, nc=nc)
        return tuple(outs)

    devices = jax.devices()[:n_cores]
    mesh = Mesh(np.asarray(devices), ("core",))
    in_specs = (PartitionSpec("core"),) * (n_params + n_outs)
    out_specs = (PartitionSpec("core"),) * n_outs
    donate = tuple(range(n_params, n_params + n_outs))
    fn = jax.jit(shard_map(_body, mesh=mesh, in_specs=in_specs,
                           out_specs=out_specs, check_rep=False),
                 donate_argnums=donate, keep_unused=True)
    return dict(fn=fn, in_names=in_names, out_names=out_names,
                out_avals=out_avals, zero_outs=zero_outs, mesh=mesh,
                n_cores=n_cores)


def _place_inputs(ex, in_maps):
    sh = NamedSharding(ex["mesh"], PartitionSpec("core"))
    n_cores = ex["n_cores"]
    return [jax.device_put(
        np.concatenate([np.asarray(in_maps[c][name]) for c in range(n_cores)], axis=0), sh)
        for name in ex["in_names"]]


def _run(ex, dev_in):
    sh = NamedSharding(ex["mesh"], PartitionSpec("core"))
    n_cores = ex["n_cores"]
    zs = [jax.device_put(np.zeros((n_cores * z.shape[0], *z.shape[1:]), z.dtype), sh)
          for z in ex["zero_outs"]]
    outs = jax.block_until_ready(ex["fn"](*dev_in, *zs))
    return [
        {name: np.asarray(outs[i]).reshape(n_cores, *ex["out_avals"][i].shape)[c]
         for i, name in enumerate(ex["out_names"])}
        for c in range(n_cores)
    ]


_CACHE = {}


def _get_compiled(prob, meta, W_pre, gamma, beta_bn, W_op, key):
    if key not in _CACHE:
        nc = build_kernel(prob, meta, W_pre, gamma, beta_bn, W_op, nloop=1)
        _CACHE[key] = _build_exec(nc, prob.n_cores)
    return _CACHE[key]


def kernel(s0=None, s1=None, x_0=None, W_pre=None, gamma=None, beta_bn=None,
           W_op=None, edge_index=None, drop_prob=None, training=None, **_ignored):
    s1 = np.asarray(s1, np.float32)
    x_0 = np.asarray(x_0, np.float32)
    W_pre = np.asarray(W_pre, np.float32)
    gamma = np.asarray(gamma, np.float32)
    beta_bn = np.asarray(beta_bn, np.float32)
    W_op = np.asarray(W_op, np.float32)
    edge_index = np.asarray(edge_index)
    N, C = s1.shape
    HID = W_pre.shape[1]
    E = edge_index.shape[1]
    prob = Prob(N, E, C, HID, n_cores=8)
    in_maps, meta = host_prep(prob, s1, x_0, edge_index)
    key = (N, E, C, HID, int(np.int64(edge_index[:, ::97]).sum()), meta["ktot"])
    ex = _get_compiled(prob, meta, W_pre, gamma, beta_bn, W_op, key)
    wins = make_weight_inputs(prob, W_pre, gamma, beta_bn, W_op)
    full_maps = [{**m, **wins} for m in in_maps]
    dev_in = _place_inputs(ex, full_maps)
    res = _run(ex, dev_in)
    out = np.concatenate(
        [unpack_out(prob, res[m]["out"])[:prob.shard] for m in range(prob.n_cores)],
        axis=0)
    return np.ascontiguousarray(out[:N]).astype(np.float32)

